# revision 12
# baseline (speedup 1.0000x reference)
"""Trainium2 Bass kernel for ComputeVecSimilarityLoss.

Reference semantics (B batches, N points, D=2):
    sm      = where(cos < th, 0, cos)                      [B,N,N]
    v[i,j]  = (gt[i] - gt[j]) * sm[i,j]  -> [B, M=N*N, D]
    dot     = v @ v^T per batch                            [B,M,M]
    idx_num = count(dot != 0)
    vabs    = sqrt(sum(v*v + 1e-9, axis=D))
    result  = sum(|dot| / (vabs_m*vabs_n)) / idx_num

Restructuring (mathematically exact, fp-equal to ~1e-6):
  * u = v / vabs  (host, O(B*M)): |dot|/(vabs_m*vabs_n) == |u_m . u_n|.
  * zero vectors are compacted away on the host; idx_num = sum_b nnz_b^2.
  * batch b -> NeuronCore b (pure data parallel, B == 8 cores).
    Each core computes S_b = sum |u_m . u_n| over its compacted M_b x M_b
    block; host does the final scalar division.

Device kernel per core (v2):
  * u is replicated by the host into 4 partition groups {0,32,64,96} so
    the PE array runs as 16 concurrent 32x32 tiles (tile_position row =
    data's partition group, col = output 32-row strip).  A second copy
    scaled by 0.5 provides the weights for diagonal blocks, so every
    PSUM value can be summed by any consumer engine without per-group
    scales (host multiplies the grand total by 2 at the end).
  * The upper triangle of the M x M |dot| matrix is covered by
    128-row x (<=512)-col blocks; each block is 4 strip-matmuls.  Blocks
    are packed bank-aligned into [128, 2048] PSUM supertiles.
  * ScalarE (Abs activation + accum_out) and VectorE (tensor_reduce with
    apply_absolute_value) split the PSUM supertiles; a final VectorE
    reduce collapses the per-consumer partials to a [128, 1] output.
"""

import os

import numpy as np

EPS = np.float32(1e-9)
N_CORES = 8
BANK = 512           # fp32 elements per PSUM bank per partition
SUPER = 2048         # PSUM supertile columns (4 banks)

LAST_RESULTS = None

_PROGRAM_CACHE = {}


def _plan(P, COLS):
    """Plan the triangular coverage.

    Returns (jobs per supertile, consumer specs).  A job is
    (row_tile, col0, width, psum_off, is_diag); every job's PSUM span
    [psum_off, psum_off+width) stays inside one 512-wide bank slot.
    A consumer spec is (tile_idx, off, k, w, engine) meaning a 3D AP
    [128, k, w] at stride BANK (k==1 -> plain 2D span) reduced by
    'engine' ('act' or 'dve').
    """
    n_tiles = P // 128

    # per row-tile: diagonal 128x128 block + right-aligned above-diagonal
    # chunks (<=512 wide, first chunk absorbs the remainder), trimmed to
    # COLS on the right edge.
    jobs = []  # (t, col0, w, is_diag)
    for t in range(n_tiles):
        d_w = min(128, max(0, COLS - 128 * t))
        if d_w > 0:
            jobs.append((t, 128 * t, d_w, True))
        s = 128 * (t + 1)
        W = COLS - s
        if W <= 0:
            continue
        w0 = W % BANK
        c = s
        if w0:
            jobs.append((t, c, w0, False))
            c += w0
        while c < COLS:
            jobs.append((t, c, BANK, False))
            c += BANK

    # row-group assignment: greedy balance of per-row-tile streamed cols
    loads = [0.0] * 1
    row_group = [0] * n_tiles
    per_t = [sum(w for (t, _, w, _) in jobs if t == tt) for tt in range(n_tiles)]
    for tt in sorted(range(n_tiles), key=lambda x: -per_t[x]):
        g = min(range(len(loads)), key=lambda x: loads[x])
        row_group[tt] = g
        loads[g] += per_t[tt]

    # PSUM packing: equal-width classes -> 512-strided slots (3D consumer),
    # small widths (<= 256) pack contiguously inside shared banks.
    by_w = {}
    for j in jobs:
        by_w.setdefault(j[2], []).append(j)

    supertiles = []  # list of lists of (t, col0, w, off, is_diag)
    consumers = []   # (tile_idx, off, k, w, engine placeholder)
    cur = []         # jobs in current supertile
    cur_off = 0

    def flush():
        nonlocal cur, cur_off
        if cur:
            supertiles.append(cur)
            cur = []
            cur_off = 0

    # big widths first (one bank slot each)
    for w in sorted(by_w, reverse=True):
        if w <= 256:
            continue
        pend = by_w.pop(w)
        while pend:
            space = (SUPER - cur_off) // BANK
            take = pend[:space] if space else []
            if not take:
                flush()
                continue
            k = len(take)
            for i, (t, c0, ww, dg) in enumerate(take):
                cur.append((t, c0, ww, cur_off + i * BANK, dg))
            consumers.append([len(supertiles), cur_off, k, w, None])
            cur_off += k * BANK
            pend = pend[len(take):]
            if cur_off >= SUPER:
                flush()
    # small widths: contiguous packing, never crossing a bank boundary
    small = [j for w in sorted(by_w, reverse=True) for j in by_w[w]]
    run_start = None
    run_k = 0
    run_w = 0
    for t, c0, w, dg in small:
        bank_rem = BANK - (cur_off % BANK)
        if w > bank_rem:
            # skip to the next bank; the gap ends the contiguous run
            if run_k:
                consumers.append([len(supertiles), run_start, 1, run_w, None])
                run_k = 0
            cur_off += bank_rem
        if cur_off + w > SUPER:
            if run_k:
                consumers.append([len(supertiles), run_start, 1, run_w, None])
                run_k = 0
            flush()
        if run_k == 0:
            run_start, run_w = cur_off, 0
        cur.append((t, c0, w, cur_off, dg))
        run_w = cur_off + w - run_start
        run_k += 1
        cur_off += w
    if run_k:
        consumers.append([len(supertiles), run_start, 1, run_w, None])
    flush()

    # engine assignment: brute-force min-makespan over the consumer list
    def act_ns(e):
        return 283.0 + (172.0 + e) / 1.2

    def dve_ns(e):
        return 1.03 * (120.0 + e) / 0.96

    n = len(consumers)
    best = (float("inf"), 0)
    for mask in range(1 << n):
        a = sum(act_ns(c[2] * c[3]) for i, c in enumerate(consumers) if mask >> i & 1)
        d = sum(dve_ns(c[2] * c[3]) for i, c in enumerate(consumers) if not mask >> i & 1)
        m = max(a, d)
        if m < best[0]:
            best = (m, mask)
    for i, c in enumerate(consumers):
        c[4] = "act" if best[1] >> i & 1 else "dve"

    # PSUM is bank-granular; round up so strided consumer views stay legal
    tile_widths = [
        -(-max(off + w for (_, _, w, off, _) in st) // BANK) * BANK
        for st in supertiles
    ]
    return supertiles, consumers, row_group, tile_widths


def _build_program(P, COLS):
    key = (P, COLS)
    if key in _PROGRAM_CACHE:
        return _PROGRAM_CACHE[key]

    import concourse.bass as bass
    import concourse.mybir as mybir
    import concourse.tile as tile
    from concourse import bacc

    f32 = mybir.dt.float32
    f16 = mybir.dt.float16
    supertiles, consumers, row_group, tile_widths = _plan(P, COLS)
    ncol = sum(c[2] if c[4] == "dve" else 1 for c in consumers)

    nc = bacc.Bacc(
        "TRN2",
        target_bir_lowering=False,
        debug=False,
        enable_asserts=False,
        num_devices=N_CORES,
    )
    # [8, 2P]: rows 2r+j hold [u_j | 0.5*u_j] for replica r
    u_dram = nc.dram_tensor("u", [6, 2 * P], f16, kind="ExternalInput")
    out_dram = nc.dram_tensor("out", [128, 1], f32, kind="ExternalOutput")

    with tile.TileContext(nc) as tc:
        with (
            tc.tile_pool(name="const", bufs=1) as const_pool,
            tc.tile_pool(name="psum", bufs=2, space="PSUM") as psum_pool,
        ):
            u = const_pool.tile([128, 2 * P], f16)
            # one plain 2-partition DMA per replica, on three parallel
            # queues (partition-strided single-DMA replication is not
            # supported: the sim flags it and hardware NRT-errors)
            for r, eng in enumerate((nc.sync,)):
                eng.dma_start(
                    u[32 * r : 32 * r + 2, :], u_dram.ap()[2 * r : 2 * r + 2, :]
                )
            partials = const_pool.tile([128, ncol], f32)
            final = const_pool.tile([128, 1], f32)

            col = 0
            cons_by_tile = {}
            for ci, (ti, off, k, w, eng) in enumerate(consumers):
                cons_by_tile.setdefault(ti, []).append((ci, off, k, w, eng, col))
                # DVE tensor_reduce emits one column per chunk; ACT
                # accum_out collapses the whole group into one column
                col += k if eng == "dve" else 1

            for ti, st in enumerate(supertiles):
                ps = psum_pool.tile([128, tile_widths[ti]], f32, tag="ps")
                for t, c0, w, off, dg in st:
                    r = row_group[t]
                    wbase = P if dg else 0  # diag weights read the 0.5*u copy
                    for c in range(4):
                        nc.tensor.matmul(
                            ps[32 * c : 32 * c + 32, off : off + w],
                            u[
                                32 * r : 32 * r + 2,
                                wbase + 128 * t + 32 * c : wbase + 128 * t + 32 * c + 32,
                            ],
                            u[32 * r : 32 * r + 2, c0 : c0 + w],
                            tile_position=(32 * r, 32 * c),
                        )
                for ci, off, k, w, eng, col0 in cons_by_tile.get(ti, []):
                    if k == 1:
                        src = ps[:, off : off + w]
                    else:
                        src = ps[:].rearrange("p (b n) -> p b n", n=BANK)[
                            :, off // BANK : off // BANK + k, 0:w
                        ]
                    if eng == "act":
                        nc.scalar.activation(
                            src,
                            src,
                            mybir.ActivationFunctionType.Abs,
                            accum_out=partials[:, col0 : col0 + 1],
                        )
                    else:
                        nc.vector.tensor_reduce(
                            partials[:, col0 : col0 + k],
                            src,
                            axis=mybir.AxisListType.X,
                            op=mybir.AluOpType.add,
                            apply_absolute_value=True,
                        )

            nc.vector.reduce_sum(final[:], partials[:], axis=mybir.AxisListType.X)
            nc.sync.dma_start(out_dram.ap(), final[:])

    nc.compile()
    _PROGRAM_CACHE[key] = nc
    return nc


def _preprocess(gt_points, cos_similarity, threshold):
    """Host O(B*N^2) prep: u vectors, compaction, replication, padding."""
    gt = np.asarray(gt_points, dtype=np.float32)
    cos = np.asarray(cos_similarity, dtype=np.float32)
    th = np.asarray(threshold, dtype=np.float32).reshape(-1)[0]
    B, N, D = gt.shape
    M = N * N

    sm = np.where(cos < th, np.float32(0), cos)
    v = ((gt[:, :, None, :] - gt[:, None, :, :]) * sm[..., None]).reshape(B, M, D)
    v = v.astype(np.float32)
    r2 = (v[..., 0] * v[..., 0] + EPS) + (v[..., 1] * v[..., 1] + EPS)
    vabs = np.sqrt(r2, dtype=np.float32)
    u = (v / vabs[..., None]).astype(np.float32)
    nz = np.any(v != 0, axis=-1)  # [B, M]
    nnz = nz.sum(axis=1).astype(np.int64)

    COLS = int(max(2, int(nnz.max())))
    P = int(-(-COLS // 128) * 128)

    in_maps = []
    for b in range(B):
        ub = u[b][nz[b]].T.astype(np.float16)  # [2, nnz_b]
        urep = np.zeros((6, 2 * P), dtype=np.float16)
        for r in range(3):
            urep[2 * r : 2 * r + 2, : ub.shape[1]] = ub
            urep[2 * r : 2 * r + 2, P : P + ub.shape[1]] = (
                ub.astype(np.float32) * 0.5
            ).astype(np.float16)
        in_maps.append({"u": urep})
    return in_maps, nnz, P, COLS


def _ensure_ntff_hook():
    """Shim antenv.axon_hooks if the image lacks it (profiling only)."""
    try:
        from antenv.axon_hooks import get_axon_ntff_profile_hook  # noqa: F401

        return
    except ImportError:
        pass

    import contextlib
    import ctypes
    import sys
    import types

    import antenv

    mod = types.ModuleType("antenv.axon_hooks")
    _state = {"hook": None}

    def set_axon_ntff_profile_hook(h):
        _state["hook"] = h

    def get_axon_ntff_profile_hook():
        return _state["hook"]

    mod.set_axon_ntff_profile_hook = set_axon_ntff_profile_hook
    mod.get_axon_ntff_profile_hook = get_axon_ntff_profile_hook
    sys.modules["antenv.axon_hooks"] = mod
    antenv.axon_hooks = mod

    so_path = "/opt/axon/libaxon_pjrt.so"
    if not os.path.exists(so_path):
        return
    lib = ctypes.CDLL(so_path)
    if not hasattr(lib, "axon_start_nrt_profile"):
        return
    lib.axon_start_nrt_profile.argtypes = [
        ctypes.POINTER(ctypes.c_int64),
        ctypes.c_size_t,
    ]
    lib.axon_start_nrt_profile.restype = ctypes.c_int64
    lib.axon_stop_nrt_profile.argtypes = [ctypes.c_char_p]
    lib.axon_stop_nrt_profile.restype = ctypes.c_int64

    @contextlib.contextmanager
    def _hook(output_dir, device_ids):
        import jax

        jax.devices()
        if device_ids:
            ids = (ctypes.c_int64 * len(device_ids))(*device_ids)
            rc = lib.axon_start_nrt_profile(ids, len(device_ids))
        else:
            rc = lib.axon_start_nrt_profile(None, 0)
        if rc != 0:
            raise RuntimeError(f"axon_start_nrt_profile rc={rc}")
        try:
            yield
        finally:
            n = lib.axon_stop_nrt_profile(str(output_dir).encode())
            if n < 0:
                raise RuntimeError(f"axon_stop_nrt_profile rc={n}")
            print(f"profile: {n} file(s) written to {output_dir}")

    set_axon_ntff_profile_hook(_hook)


def kernel(gt_points, cos_similarity, threshold):
    global LAST_RESULTS
    in_maps, nnz, P, COLS = _preprocess(gt_points, cos_similarity, threshold)
    B = len(in_maps)

    total_count = int((nnz.astype(np.int64) ** 2).sum())
    if total_count == 0:
        with np.errstate(invalid="ignore", divide="ignore"):
            return (np.float32(0) / np.float32(0)).astype(np.float32)

    from concourse.bass_utils import run_bass_kernel_spmd

    nc = _build_program(P, COLS)
    assert B <= N_CORES, "one batch per core"
    trace = os.environ.get("KERNEL_TRACE", "") not in ("", "0")
    if trace:
        _ensure_ntff_hook()
    res = run_bass_kernel_spmd(
        nc,
        in_maps,
        core_ids=list(range(B)),
        trace=trace,
    )
    LAST_RESULTS = res

    total = 0.0
    for b in range(B):
        out = res.results[b]["out"]
        # partials hold (upper + 0.5*diag); x2 recovers the full sum
        total += 2.0 * float(np.sum(out, dtype=np.float64))

    return np.asarray(
        np.float32(total) / np.float32(total_count), dtype=np.float32
    )
